# revision 30
# baseline (speedup 1.0000x reference)
"""Trainium2 Bass kernel for the GNN message-passing model.

Strategy: pure data-parallel over batch (B=16 -> 2 batches per core, 8 cores,
no cross-core communication).

Key design points vs the earlier baseline:
  * The whole adjacency (transposed) is kept RESIDENT in SBUF as fp8-e4m3
    (16 MiB), scaled by 2^19 so values land in e4m3's normal range. This
    removes the per-layer HBM re-streaming (~96 MiB/core) that caused DMA
    waits and HAM clock-throttle oscillation.
  * The adjacency matmuls (the dominant cost) run in fp8 DoubleRow perf mode:
    contraction pairs are packed 2-per-cell, halving the number of
    accumulation passes (16 instead of 32 per 512-col output tile).
  * Batch 1's feature order is PERMUTED (cut features moved to partition
    rows 64:106/114) so both batches' cut features live in one SBUF tile and
    a single PE transpose per 128-node block serves both batches
    (32 transposes/layer instead of 64).
  * Layer-0 feature matmul is fused into the positional front-end chunk loop
    so h1/h2 activations never need full-length SBUF tiles.

Weight-only folds done on host (pure parameter preprocessing):
  W3fold = pw3 @ gw0[100:200]   (positional-MLP last layer folded into gw0)
  t4     = emb @ gw0[200:300]   (embedding table folded into gw0)
  pb3f   = pb3 @ gw0[100:200]   (bias fold)
mask_idx is re-encoded as a one-hot (4 classes) so the embedding lookup
becomes a K=4 matmul accumulated into the same PSUM as the layer-0 matmul.
"""

import numpy as np
import ml_dtypes

import concourse.bass as bass
import concourse.mybir as mybir
import concourse.tile as tile
from concourse.masks import make_identity
from concourse.bass_utils import run_bass_kernel_spmd

F32 = mybir.dt.float32
BF16 = mybir.dt.bfloat16
FP8 = mybir.dt.float8e4
AF = mybir.ActivationFunctionType
BF = ml_dtypes.bfloat16
E4 = ml_dtypes.float8_e4m3

B, N, BC = 16, 4096, 2          # batches, nodes, batches per core
NCORES = 8
NB = N // 512                   # 8 column blocks of 512
NQ = N // 128                   # 32 contraction sub-blocks of 128
MAGIC = float(1.5 * 2 ** 23)    # fp32 round-to-nearest magic constant
TWO_PI = float(2.0 * np.pi)
SA = float(2.0 ** 19)           # adjacency fp8 scale
SF = 64.0                       # cut-feature fp8 scale
INV = float(1.0 / (SA * SF))    # undo scale after the adjacency matmul
MST = 128                       # fcst stationary slot stride (bytes, fp8)
DR = mybir.MatmulPerfMode.DoubleRow

run_kwargs = {}                 # test.py may inject trace kwargs here


def split_excess_waits(nc, max_waits=1):
    """Walrus codegen on this image rejects >1 sem wait per instruction;
    move excess waits onto preceding same-engine no-ops."""
    n_split = 0
    for fn in nc.m.functions:
        for blk in fn.blocks:
            insts = list(blk.instructions)
            out = []
            changed = False
            for inst in insts:
                si = getattr(inst, "sync_info", None)
                if si is not None and len(si.on_wait) > max_waits:
                    waits = list(si.on_wait)
                    chunks = [waits[i:i + max_waits]
                              for i in range(0, len(waits), max_waits)]
                    for ci, ch in enumerate(chunks[:-1]):
                        nop = mybir.InstNoOp(
                            name=f"{inst.name}-wsplit-{ci}", ins=[], outs=[])
                        nop.engine = inst.engine
                        nop.sync_info = mybir.SyncInfo(on_wait=ch, on_update=[])
                        out.append(nop)
                        n_split += 1
                    inst.sync_info = mybir.SyncInfo(
                        on_wait=chunks[-1], on_update=list(si.on_update))
                    changed = True
                out.append(inst)
            if changed:
                blk.instructions = out
    return n_split


def _param(nc, name, shape, dt):
    return nc.declare_dram_parameter(name, list(shape), dt, isOutput=False)


def build_bass(split=True):
    nc = bass.Bass()

    adjq = _param(nc, "adjq", [N, N], FP8)
    meshTd = _param(nc, "meshTd", [BC, 3, N], F32)
    meshTb = _param(nc, "meshTb", [BC, 3, N], BF16)
    onehotd = _param(nc, "onehotd", [4, BC * N], BF16)
    maskTd = _param(nc, "maskTd", [50, BC], F32)

    pw1p = _param(nc, "pw1p", [67, 25], BF16)
    pw2d = _param(nc, "pw2d", [25, 50], BF16)
    w3fold = _param(nc, "w3fold", [50, 128], BF16)
    w3foldp = _param(nc, "w3foldp", [50, 128], BF16)
    t4d = _param(nc, "t4d", [4, 128], BF16)
    t4pd = _param(nc, "t4pd", [4, 128], BF16)
    gw1d = _param(nc, "gw1d", [128, 128], BF16)
    gw1pd = _param(nc, "gw1pd", [128, 128], BF16)
    gw2d = _param(nc, "gw2d", [128, 128], BF16)
    gw2pd = _param(nc, "gw2pd", [128, 128], BF16)
    gw3d = _param(nc, "gw3d", [128, 50], BF16)
    gw3pd = _param(nc, "gw3pd", [128, 50], BF16)
    aw1ad = _param(nc, "aw1ad", [50, 128], F32)
    aw1bd = _param(nc, "aw1bd", [50, 72], F32)
    aw2ad = _param(nc, "aw2ad", [128, 100], F32)
    aw2bd = _param(nc, "aw2bd", [72, 100], F32)
    aw3d = _param(nc, "aw3d", [100, 100], F32)
    gw0Ld = _param(nc, "gw0Ld", [100, 128], F32)
    gw0Lpd = _param(nc, "gw0Lpd", [100, 128], F32)
    selfd = _param(nc, "selfd", [6, 62], F32)
    biasd = _param(nc, "biasd", [128, 12], F32)
    # bias columns: 0 ab1a[128], 1 ab1b[72], 2 ab2[100], 3 ab3[100],
    # 4 pb3f[128], 5 pb3f_perm[128], 6 pb1[25], 7 pb2[50],
    # 8/9/10 gb{0,1,2}cut (rows 0:42 + 64:106), 11 gb3 (rows 0:50 + 64:114)
    outd = nc.declare_dram_parameter("outd", [128, 1], F32, isOutput=True)

    with tile.TileContext(nc) as tc:
        _emit(nc, tc, locals())
    if split:
        split_excess_waits(nc)
    return nc


def _emit(nc, tc, d):
    import contextlib
    ctx = contextlib.ExitStack()
    adjq, meshTd, onehotd, maskTd = d["adjq"], d["meshTd"], d["onehotd"], d["maskTd"]
    biasd, outd = d["biasd"], d["outd"]

    meshTb = d["meshTb"]
    cpool = ctx.enter_context(tc.tile_pool(name="consts", bufs=1))
    resp = ctx.enter_context(tc.tile_pool(name="resadj", bufs=1))
    actp = ctx.enter_context(tc.tile_pool(name="acts", bufs=1))
    smallp = ctx.enter_context(tc.tile_pool(name="small", bufs=2))
    dvep = ctx.enter_context(tc.tile_pool(name="dvework", bufs=1))

    ps_misc = ctx.enter_context(tc.tile_pool(name="psmisc", bufs=2, space="PSUM"))
    ps_tp = ctx.enter_context(tc.tile_pool(name="pstp", bufs=2, space="PSUM"))
    ps_left = ctx.enter_context(tc.tile_pool(name="psleft", bufs=4, space="PSUM"))

    # ---------------- resident fp8 adjacency (sync queue) ----------------
    adjr = adjq[:].rearrange("(q p) c -> p q c", p=128)   # [128, 32, 4096]
    adjs = resp.tile([128, NQ * N], FP8, tag="adjs", name="adjs")
    adjs3 = adjs[:].rearrange("p (q c) -> p q c", c=N)
    for ib in range(NB):
        cs = slice(ib * 512, (ib + 1) * 512)
        nc.sync.dma_start(out=adjs3[:, :, cs], in_=adjr[:, :, cs])

    # ---------------- constants (gpsimd queue) ----------------
    def ctile(dram, shape, dt):
        nm = f"c_{dram.name}"
        t = cpool.tile(list(shape), dt, tag=nm, name=nm)
        nc.gpsimd.dma_start(out=t[:], in_=dram[:])
        return t

    ident = cpool.tile([128, 128], BF16)
    make_identity(nc, ident[:])
    aw1a = ctile(d["aw1ad"], [50, 128], F32)
    aw1b = ctile(d["aw1bd"], [50, 72], F32)
    aw2a = ctile(d["aw2ad"], [128, 100], F32)
    aw2b = ctile(d["aw2bd"], [72, 100], F32)
    aw3 = ctile(d["aw3d"], [100, 100], F32)
    gw0L = [ctile(d["gw0Ld"], [100, 128], F32),
            ctile(d["gw0Lpd"], [100, 128], F32)]
    selfreq = ctile(d["selfd"], [6, 62], F32)
    biases = ctile(biasd, [128, 12], F32)
    maskT = ctile(maskTd, [50, BC], F32)
    pw1 = ctile(d["pw1p"], [67, 25], BF16)
    pw2 = ctile(d["pw2d"], [25, 50], BF16)
    w3f = [ctile(d["w3fold"], [50, 128], BF16),
           ctile(d["w3foldp"], [50, 128], BF16)]
    # t4/onehot live at partitions 64:68 so the K=4 embedding matmul can run
    # row-tiled concurrently with the K=50 w3fold matmul.
    t4 = []
    for b in range(BC):
        t4t = cpool.tile([68, 128], BF16, tag=f"t4_{b}", name=f"t4_{b}")
        nc.gpsimd.dma_start(out=t4t[64:68, :], in_=d["t4d" if b == 0 else "t4pd"][:])
        t4.append(t4t)
    onehot = cpool.tile([68, BC * N], BF16, tag="onehot", name="onehot")
    nc.gpsimd.dma_start(out=onehot[64:68, :], in_=onehotd[:])
    gws = {1: [ctile(d["gw1d"], [128, 128], BF16),
               ctile(d["gw1pd"], [128, 128], BF16)],
           2: [ctile(d["gw2d"], [128, 128], BF16),
               ctile(d["gw2pd"], [128, 128], BF16)],
           3: [ctile(d["gw3d"], [128, 50], BF16),
               ctile(d["gw3pd"], [128, 50], BF16)]}

    def bcol(col, p0, p1):
        return biases[p0:p1, col:col + 1]

    # ---------------- activation tiles ----------------
    xt = actp.tile([128, BC * N], BF16, tag="x")           # [feat, b*N+n]
    fcutT = actp.tile([114, N], BF16, tag="fcutT")         # b0 rows 0:50, b1 64:114
    nc.gpsimd.memset(fcutT[:], 0.0)
    fcst = actp.tile([128, NQ * MST], FP8, tag="fcst")     # stationary slots
    nc.gpsimd.memset(fcst[:], 0.0)
    cvec = actp.tile([128, BC], F32, tag="cvec")
    cvecs = actp.tile([128, BC], F32, tag="cvecs")
    mx = actp.tile([128, NB], F32, tag="mx")
    nc.gpsimd.memset(mx[:], -1e30)
    outsb = actp.tile([128, 1], F32, tag="outsb")
    fcst3 = fcst[:].rearrange("p (q m) -> p q m", m=MST)

    # ---------------- action MLP (tiny, fp32) ----------------
    pa = ps_misc.tile([128, 2], F32, tag="misc")
    nc.tensor.matmul(pa[:], lhsT=aw1a[:], rhs=maskT[:], start=True, stop=True)
    a1a = smallp.tile([128, 2], F32, tag="a1a")
    nc.scalar.activation(a1a[:], pa[:], AF.Relu, bias=bcol(0, 0, 128))
    pb = ps_misc.tile([72, 2], F32, tag="misc")
    nc.tensor.matmul(pb[:], lhsT=aw1b[:], rhs=maskT[:], start=True, stop=True)
    a1b = smallp.tile([72, 2], F32, tag="a1b")
    nc.scalar.activation(a1b[:], pb[:], AF.Relu, bias=bcol(1, 0, 72))
    pc = ps_misc.tile([100, 2], F32, tag="misc")
    nc.tensor.matmul(pc[:], lhsT=aw2a[:], rhs=a1a[:], start=True, stop=False)
    nc.tensor.matmul(pc[:], lhsT=aw2b[:], rhs=a1b[:], start=False, stop=True)
    a2 = smallp.tile([100, 2], F32, tag="a2")
    nc.scalar.activation(a2[:], pc[:], AF.Relu, bias=bcol(2, 0, 100))
    pd = ps_misc.tile([100, 2], F32, tag="misc")
    nc.tensor.matmul(pd[:], lhsT=aw3[:], rhs=a2[:], start=True, stop=True)
    a3 = smallp.tile([100, 2], F32, tag="a3")
    nc.scalar.activation(a3[:], pd[:], AF.Identity, bias=bcol(3, 0, 100))
    pe_ = ps_misc.tile([128, 2], F32, tag="misc")
    for b in range(BC):
        nc.tensor.matmul(pe_[:, b:b + 1], lhsT=gw0L[b][:], rhs=a3[:, b:b + 1],
                         start=True, stop=True)
        nc.scalar.activation(cvec[:, b:b + 1], pe_[:, b:b + 1], AF.Identity,
                             bias=bcol(4 + b, 0, 128))
    nc.scalar.activation(cvecs[:], cvec[:], AF.Identity, scale=SF)

    # ---------------- positional front-end + fused layer-0 features -------
    # batch-1 cut rows live at partitions 64:106 via the host-side column
    # permutation of w3foldp/t4pd; right-part rows land at 0:64 and 106:128.
    for ch in range(NB):
        cs = slice(ch * 512, (ch + 1) * 512)
        m6 = smallp.tile([6, 512], F32, tag="m6")
        nc.scalar.dma_start(
            out=m6[:],
            in_=meshTd[:, :, cs].rearrange("b c n -> (b c) n"))
        # t[30b+k, n] = mesh[b, k%3, n] * freq[k//3] / (2*pi)
        t2 = ps_misc.tile([62, 512], F32, tag="misc")
        nc.tensor.matmul(t2[:], lhsT=selfreq[:], rhs=m6[:], start=True, stop=True)
        # range reduction: d = t - round(t); dc = (t+0.25) - round(t+0.25)
        # PSUM-sourced ops on DVE, SBUF-only ops on gpsimd.
        r1 = dvep.tile([62, 512], F32, tag="r1")
        nc.vector.tensor_scalar_add(r1[:], t2[:], MAGIC)
        r2 = dvep.tile([62, 512], F32, tag="r2")
        nc.gpsimd.tensor_scalar_add(r2[:], r1[:], -MAGIC)
        dd = dvep.tile([62, 512], F32, tag="dd")
        nc.vector.tensor_sub(dd[:], t2[:], r2[:])
        tcq = dvep.tile([62, 512], F32, tag="tcq")
        nc.vector.tensor_scalar_add(tcq[:], t2[:], 0.25)
        nc.gpsimd.tensor_scalar_add(r1[:], tcq[:], MAGIC)
        nc.gpsimd.tensor_scalar_add(r2[:], r1[:], -MAGIC)
        dc = dvep.tile([62, 512], F32, tag="dc")
        nc.gpsimd.tensor_sub(dc[:], tcq[:], r2[:])
        for b in range(BC):
            xs = slice(b * N + ch * 512, b * N + (ch + 1) * 512)
            pein = smallp.tile([67, 512], BF16, tag=f"pein{b}")
            nc.vector.memset(pein[:], 0.0)
            nc.scalar.activation(pein[0:30, :], dd[32 * b:32 * b + 30, :],
                                 AF.Sin, scale=TWO_PI)
            nc.scalar.activation(pein[32:62, :], dc[32 * b:32 * b + 30, :],
                                 AF.Sin, scale=TWO_PI)
            nc.scalar.dma_start(out=pein[64:67, :], in_=meshTb[b, :, cs])
            # h1 = relu(pe_in @ pw1 + pb1)
            ph1 = ps_tp.tile([25, 512], F32, tag="tp")
            nc.tensor.matmul(ph1[:], lhsT=pw1[:], rhs=pein[:],
                             start=True, stop=True)
            h1 = smallp.tile([25, 512], BF16, tag=f"h1{b}")
            nc.scalar.activation(h1[:], ph1[:], AF.Relu, bias=bcol(6, 0, 25))
            # h2 = relu(h1 @ pw2 + pb2)
            ph2 = ps_tp.tile([50, 512], F32, tag="tp")
            nc.tensor.matmul(ph2[:], lhsT=pw2[:], rhs=h1[:],
                             start=True, stop=True)
            h2 = smallp.tile([50, 512], BF16, tag=f"h2{b}")
            nc.vector.tensor_scalar(h2[:], ph2[:], bcol(7, 0, 50), 0.0,
                                    mybir.AluOpType.add, mybir.AluOpType.max)
            # layer-0 features: f0 = [h2 | onehot] @ [w3fold; t4] (+ cvec);
            # the K=50 and K=4 matmuls run row-tiled concurrently.
            pf = ps_misc.tile([128, 512], F32, tag="misc")
            nc.tensor.matmul(pf[:], lhsT=w3f[b][:], rhs=h2[:],
                             start=True, stop=False)
            nc.tensor.matmul(pf[:], lhsT=t4[b][64:68, :],
                             rhs=onehot[64:68, xs],
                             start=False, stop=True)
            # full-tile relu (DVE): rows overlapping the cut range get
            # garbage and are overwritten by the C drain later.
            nc.vector.tensor_scalar(xt[:, xs], pf[:, :],
                                    cvec[:, b:b + 1], 0.0,
                                    mybir.AluOpType.add, mybir.AluOpType.max)
            if b == 0:
                nc.scalar.activation(fcutT[0:42, cs], pf[0:42, :],
                                     AF.Identity, bias=cvecs[0:42, 0:1],
                                     scale=SF)
            else:
                nc.scalar.activation(fcutT[64:106, cs], pf[64:106, :],
                                     AF.Identity, bias=cvecs[64:106, 1:2],
                                     scale=SF)

    # ---------------- GCN layers ----------------
    for li in range(4):
        last = li == 3
        cd = 50 if last else 42
        mm = 64 + cd                       # stationary packed width
        # phase A: f = x @ gw (skipped for li=0: fused above)
        if li > 0:
            for ch in range(NB):
                if last:
                    pf = ps_misc.tile([128, 512], F32, tag="misc")
                    for b in range(BC):
                        xs = slice(b * N + ch * 512, b * N + (ch + 1) * 512)
                        nc.tensor.matmul(pf[64 * b:64 * b + 50, :],
                                         lhsT=gws[3][b][:], rhs=xt[:, xs],
                                         start=True, stop=True)
                    cs = slice(ch * 512, (ch + 1) * 512)
                    nc.scalar.activation(fcutT[0:50, cs], pf[0:50, :],
                                         AF.Identity, scale=SF)
                    nc.scalar.activation(fcutT[64:114, cs], pf[64:114, :],
                                         AF.Identity, scale=SF)
                else:
                    for b in range(BC):
                        xs = slice(b * N + ch * 512, b * N + (ch + 1) * 512)
                        cs = slice(ch * 512, (ch + 1) * 512)
                        pf = ps_misc.tile([128, 512], F32, tag="misc")
                        nc.tensor.matmul(pf[:], lhsT=gws[li][b][:],
                                         rhs=xt[:, xs], start=True, stop=True)
                        # full-tile relu; cut rows get garbage and are
                        # rewritten by the C drain.
                        if b == 0:
                            nc.scalar.activation(xt[:, xs], pf[:, :], AF.Relu)
                        else:
                            nc.vector.tensor_scalar_max(xt[:, xs], pf[:, :],
                                                        0.0)
                        if b == 0:
                            nc.scalar.activation(fcutT[0:42, cs], pf[0:42, :],
                                                 AF.Identity, scale=SF)
                        else:
                            nc.scalar.activation(fcutT[64:106, cs],
                                                 pf[64:106, :],
                                                 AF.Identity, scale=SF)
        # phase B: one transpose per 128-node block serves both batches
        for q in range(NQ):
            jc = slice(q * 128, (q + 1) * 128)
            tp = ps_tp.tile([128, 128], BF16, tag="tp")
            nc.tensor.transpose(tp[:, 0:mm], fcutT[0:mm, jc],
                                ident[0:mm, 0:mm])
            if q % 2 == 0:
                nc.vector.tensor_copy(fcst3[:, q, 0:mm], tp[:, 0:mm])
            else:
                nc.scalar.activation(fcst3[:, q, 0:mm], tp[:, 0:mm],
                                     AF.Identity)
        # phase C: left^T = fcst.T @ adjT in fp8 DoubleRow, jt pairs of q.
        # li=0 uses pairs so compute can start before the whole adjacency
        # has landed in SBUF; later layers use groups of 4 (fewer PSUM swaps).
        if li == 0:
            groups = [(0, 2), (2, 4), (4, 6), (6, 8)]
        else:
            groups = [(0, 4), (4, 8)]
        for g0, g1 in groups:
            pls = {}
            for ib in range(g0, g1):
                pls[ib] = ps_left.tile([mm, 512], F32, tag="left",
                                       name=f"pl{li}_{ib}")
            for jt in range(NQ // 2):
                lhsT = fcst3[:, 2 * jt:2 * jt + 2, 0:mm]
                for ib in range(g0, g1):
                    rhs = adjs3[:, 2 * jt:2 * jt + 2,
                                ib * 512:(ib + 1) * 512]
                    nc.tensor.matmul(pls[ib][:], lhsT=lhsT, rhs=rhs,
                                     start=(jt == 0), stop=(jt == NQ // 2 - 1),
                                     perf_mode=DR)
            for ib in range(g0, g1):
                pl = pls[ib]
                if not last:
                    # xt keeps the (SA*SF)-scaled cut values; the next-layer
                    # gw cut rows are pre-divided on the host, and the gb cut
                    # biases are pre-multiplied (cols 8-10).
                    nc.vector.tensor_scalar(
                        xt[0:42, ib * 512:(ib + 1) * 512],
                        pl[0:42, :], bcol(8 + li, 0, 42), 0.0,
                        mybir.AluOpType.add, mybir.AluOpType.max)
                    nc.vector.tensor_scalar(
                        xt[64:106, N + ib * 512:N + (ib + 1) * 512],
                        pl[64:106, :], bcol(8 + li, 64, 106), 0.0,
                        mybir.AluOpType.add, mybir.AluOpType.max)
                else:
                    nc.vector.tensor_reduce(
                        mx[0:50, ib:ib + 1], pl[0:50, :],
                        mybir.AxisListType.X, mybir.AluOpType.max)
                    nc.vector.tensor_reduce(
                        mx[64:114, ib:ib + 1], pl[64:114, :],
                        mybir.AxisListType.X, mybir.AluOpType.max)

    # ---------------- final max + bias + output ----------------
    mxr = smallp.tile([128, 1], F32, tag="mxr")
    nc.vector.tensor_reduce(mxr[:], mx[:], mybir.AxisListType.X,
                            mybir.AluOpType.max)
    nc.scalar.activation(outsb[:], mxr[:], AF.Identity, bias=bcol(11, 0, 128),
                         scale=INV)
    nc.sync.dma_start(out=outd[:], in_=outsb[:])
    ctx.close()


# ---------------------------------------------------------------------------
# host side
# ---------------------------------------------------------------------------

# batch-1 feature permutation: rows 0:64 <- features 42:106,
# rows 64:106 <- features 0:42 (the cut), rows 106:128 <- features 106:128
PI = np.concatenate([np.arange(42, 106), np.arange(0, 42),
                     np.arange(106, 128)]).astype(np.int64)


def _prep_shared(inp):
    """Host preprocessing shared across cores (weights + adj)."""
    f32 = np.float32
    adjq = np.ascontiguousarray(
        inp["adj"].astype(f32).T * f32(SA)).astype(E4)

    gw0 = inp["gw0"].astype(f32)
    w3fold = (inp["pw3"].astype(f32) @ gw0[100:200])
    t4 = (inp["emb"].astype(f32) @ gw0[200:300])
    pb3f = (inp["pb3"].astype(f32) @ gw0[100:200]).astype(f32)
    gw0L = np.ascontiguousarray(gw0[:100])

    # pe_in row permutation: ours = [sin(f,c) x30 | cos(f,c) x30 | mesh x3]
    pw1f = inp["pw1"].astype(f32)
    pw1p_ = np.zeros((67, 25), f32)
    for k in range(30):
        f, c = divmod(k, 3)
        pw1p_[k] = pw1f[f * 6 + c]          # sin rows
        pw1p_[32 + k] = pw1f[f * 6 + 3 + c]  # cos rows
    pw1p_[64:67] = pw1f[60:63]
    pw1p = pw1p_.astype(BF)

    freqs = np.asarray([np.pi] + [2.0 * np.pi * i for i in range(1, 10)], f32)
    freq2 = np.repeat(freqs, 3) / (2.0 * np.pi)   # [30]
    self6 = np.zeros((6, 62), f32)
    for b in range(2):
        for k in range(30):
            self6[3 * b + k % 3, 32 * b + k] = freq2[k]

    # xt carries the adjacency-matmul output still scaled by SA*SF; undo the
    # scale by pre-dividing the gw rows that consume cut features (b0 rows
    # 0:42, permuted-b1 rows 64:106) and pre-multiplying the gb cut biases.
    gw1 = inp["gw1"].astype(f32)
    gw2 = inp["gw2"].astype(f32)
    gw3 = inp["gw3"].astype(f32)
    gw1p = gw1[np.ix_(PI, PI)].copy()
    gw2p = gw2[np.ix_(PI, PI)].copy()
    gw3p = gw3[PI, :].copy()
    for g in (gw1, gw2, gw3):
        g[0:42] *= f32(INV)
    for g in (gw1p, gw2p, gw3p):
        g[64:106] *= f32(INV)

    biasd = np.zeros((128, 12), f32)
    biasd[0:128, 0] = inp["ab1"][:128]
    biasd[0:72, 1] = inp["ab1"][128:200]
    biasd[0:100, 2] = inp["ab2"]
    biasd[0:100, 3] = inp["ab3"]
    biasd[0:128, 4] = pb3f
    biasd[0:128, 5] = pb3f[PI]
    biasd[0:25, 6] = inp["pb1"].astype(f32)
    biasd[0:50, 7] = inp["pb2"].astype(f32)
    for li in range(3):
        gb = inp[f"gb{li}"].astype(f32) * f32(SA * SF)
        biasd[0:42, 8 + li] = gb[:42]
        biasd[64:106, 8 + li] = gb[:42]
    gb3 = inp["gb3"].astype(f32)
    biasd[0:50, 11] = gb3
    biasd[64:114, 11] = gb3

    return {
        "adjq": adjq,
        "pw1p": pw1p,
        "pw2d": inp["pw2"].astype(BF),
        "w3fold": w3fold.astype(BF),
        "w3foldp": np.ascontiguousarray(w3fold[:, PI]).astype(BF),
        "t4d": t4.astype(BF),
        "t4pd": np.ascontiguousarray(t4[:, PI]).astype(BF),
        "gw1d": gw1.astype(BF),
        "gw1pd": np.ascontiguousarray(gw1p).astype(BF),
        "gw2d": gw2.astype(BF),
        "gw2pd": np.ascontiguousarray(gw2p).astype(BF),
        "gw3d": gw3.astype(BF),
        "gw3pd": np.ascontiguousarray(gw3p).astype(BF),
        "aw1ad": np.ascontiguousarray(inp["aw1"].astype(f32)[:, :128]),
        "aw1bd": np.ascontiguousarray(inp["aw1"].astype(f32)[:, 128:200]),
        "aw2ad": np.ascontiguousarray(inp["aw2"].astype(f32)[:128]),
        "aw2bd": np.ascontiguousarray(inp["aw2"].astype(f32)[128:200]),
        "aw3d": inp["aw3"].astype(f32),
        "gw0Ld": gw0L,
        "gw0Lpd": np.ascontiguousarray(gw0L[:, PI]),
        "selfd": self6,
        "biasd": biasd,
    }


def _prep_core(inp, shared, core):
    bs = slice(core * BC, (core + 1) * BC)
    f32 = np.float32
    mesh = inp["mesh"].astype(f32)[bs]                       # [2, N, 3]
    meshT = np.ascontiguousarray(mesh.transpose(0, 2, 1))    # [2, 3, N]
    mi = inp["mask_idx"][bs]                                 # [2, N] int32
    onehot = (mi[:, None, :] == np.arange(4, dtype=mi.dtype)[None, :, None])
    onehot = np.ascontiguousarray(
        onehot.transpose(1, 0, 2).reshape(4, BC * N)).astype(BF)
    maskT = np.ascontiguousarray(inp["mask"].astype(f32)[bs].T)  # [50, 2]
    m = dict(shared)
    m["meshTd"] = meshT
    m["meshTb"] = meshT.astype(BF)
    m["onehotd"] = onehot
    m["maskTd"] = maskT
    return m


_CACHED = {}


def kernel(**inputs) -> np.ndarray:
    if "nc" not in _CACHED:
        _CACHED["nc"] = build_bass()
    nc = _CACHED["nc"]
    shared = _prep_shared(inputs)
    in_maps = [_prep_core(inputs, shared, c) for c in range(NCORES)]
    res = run_bass_kernel_spmd(nc, in_maps, list(range(NCORES)), **run_kwargs)
    out = np.empty((B, 50), np.float32)
    for c in range(NCORES):
        o = res.results[c]["outd"][:, 0]
        out[2 * c] = o[0:50]
        out[2 * c + 1] = o[64:114]
    _CACHED["last_results"] = res
    return out


# revision 33
# speedup vs baseline: 1.5011x; 1.5011x over previous
"""Trainium2 Bass kernel for the GNN message-passing model.

Strategy: pure data-parallel over batch (B=16 -> 2 batches per core, 8 cores,
no cross-core communication).

Key design points vs the earlier baseline:
  * The whole adjacency (transposed) is kept RESIDENT in SBUF as fp8-e4m3
    (16 MiB), scaled by 2^19 so values land in e4m3's normal range. This
    removes the per-layer HBM re-streaming (~96 MiB/core) that caused DMA
    waits and HAM clock-throttle oscillation.
  * The adjacency matmuls (the dominant cost) run in fp8 DoubleRow perf mode:
    contraction pairs are packed 2-per-cell, halving the number of
    accumulation passes (16 instead of 32 per 512-col output tile).
  * Batch 1's feature order is PERMUTED (cut features moved to partition
    rows 64:106/114) so both batches' cut features live in one SBUF tile and
    a single PE transpose per 128-node block serves both batches
    (32 transposes/layer instead of 64).
  * Layer-0 feature matmul is fused into the positional front-end chunk loop
    so h1/h2 activations never need full-length SBUF tiles.

Weight-only folds done on host (pure parameter preprocessing):
  W3fold = pw3 @ gw0[100:200]   (positional-MLP last layer folded into gw0)
  t4     = emb @ gw0[200:300]   (embedding table folded into gw0)
  pb3f   = pb3 @ gw0[100:200]   (bias fold)
mask_idx is re-encoded as a one-hot (4 classes) so the embedding lookup
becomes a K=4 matmul accumulated into the same PSUM as the layer-0 matmul.
"""

import numpy as np
import ml_dtypes

import concourse.bass as bass
import concourse.mybir as mybir
import concourse.tile as tile
from concourse.masks import make_identity
from concourse.bass_utils import run_bass_kernel_spmd

F32 = mybir.dt.float32
BF16 = mybir.dt.bfloat16
FP8 = mybir.dt.float8e4
AF = mybir.ActivationFunctionType
BF = ml_dtypes.bfloat16
E4 = ml_dtypes.float8_e4m3

B, N, BC = 16, 4096, 2          # batches, nodes, batches per core
NCORES = 8
NB = N // 512                   # 8 column blocks of 512
NQ = N // 128                   # 32 contraction sub-blocks of 128
MAGIC = float(1.5 * 2 ** 23)    # fp32 round-to-nearest magic constant
TWO_PI = float(2.0 * np.pi)
SA = float(2.0 ** 19)           # adjacency fp8 scale
SF = 64.0                       # cut-feature fp8 scale
INV = float(1.0 / (SA * SF))    # undo scale after the adjacency matmul
MST = 128                       # fcst stationary slot stride (bytes, fp8)
DR = mybir.MatmulPerfMode.DoubleRow

run_kwargs = {}                 # test.py may inject trace kwargs here


def split_excess_waits(nc, max_waits=1):
    """Walrus codegen on this image rejects >1 sem wait per instruction;
    move excess waits onto preceding same-engine no-ops."""
    n_split = 0
    for fn in nc.m.functions:
        for blk in fn.blocks:
            insts = list(blk.instructions)
            out = []
            changed = False
            for inst in insts:
                si = getattr(inst, "sync_info", None)
                if si is not None and len(si.on_wait) > max_waits:
                    waits = list(si.on_wait)
                    chunks = [waits[i:i + max_waits]
                              for i in range(0, len(waits), max_waits)]
                    for ci, ch in enumerate(chunks[:-1]):
                        nop = mybir.InstNoOp(
                            name=f"{inst.name}-wsplit-{ci}", ins=[], outs=[])
                        nop.engine = inst.engine
                        nop.sync_info = mybir.SyncInfo(on_wait=ch, on_update=[])
                        out.append(nop)
                        n_split += 1
                    inst.sync_info = mybir.SyncInfo(
                        on_wait=chunks[-1], on_update=list(si.on_update))
                    changed = True
                out.append(inst)
            if changed:
                blk.instructions = out
    return n_split


def _param(nc, name, shape, dt):
    return nc.declare_dram_parameter(name, list(shape), dt, isOutput=False)


def build_bass(split=True):
    nc = bass.Bass()

    adjq = _param(nc, "adjq", [N, N], FP8)
    meshTd = _param(nc, "meshTd", [BC, 3, N], F32)
    meshTb = _param(nc, "meshTb", [BC, 3, N], BF16)
    onehotd = _param(nc, "onehotd", [4, BC * N], BF16)
    maskTd = _param(nc, "maskTd", [50, BC], F32)

    pw1p = _param(nc, "pw1p", [67, 25], BF16)
    pw2d = _param(nc, "pw2d", [25, 50], BF16)
    w3fold = _param(nc, "w3fold", [50, 128], BF16)
    w3foldp = _param(nc, "w3foldp", [50, 128], BF16)
    t4d = _param(nc, "t4d", [4, 128], BF16)
    t4pd = _param(nc, "t4pd", [4, 128], BF16)
    gw1d = _param(nc, "gw1d", [128, 128], BF16)
    gw1pd = _param(nc, "gw1pd", [128, 128], BF16)
    gw2d = _param(nc, "gw2d", [128, 128], BF16)
    gw2pd = _param(nc, "gw2pd", [128, 128], BF16)
    gw3d = _param(nc, "gw3d", [128, 50], BF16)
    gw3pd = _param(nc, "gw3pd", [128, 50], BF16)
    aw1ad = _param(nc, "aw1ad", [50, 128], F32)
    aw1bd = _param(nc, "aw1bd", [50, 72], F32)
    aw2ad = _param(nc, "aw2ad", [128, 100], F32)
    aw2bd = _param(nc, "aw2bd", [72, 100], F32)
    aw3d = _param(nc, "aw3d", [100, 100], F32)
    gw0Ld = _param(nc, "gw0Ld", [100, 128], F32)
    gw0Lpd = _param(nc, "gw0Lpd", [100, 128], F32)
    selfd = _param(nc, "selfd", [6, 62], F32)
    biasd = _param(nc, "biasd", [128, 12], F32)
    # bias columns: 0 ab1a[128], 1 ab1b[72], 2 ab2[100], 3 ab3[100],
    # 4 pb3f[128], 5 pb3f_perm[128], 6 pb1[25], 7 pb2[50],
    # 8/9/10 gb{0,1,2}cut (rows 0:42 + 64:106), 11 gb3 (rows 0:50 + 64:114)
    outd = nc.declare_dram_parameter("outd", [128, 1], F32, isOutput=True)

    with tile.TileContext(nc) as tc:
        _emit(nc, tc, locals())
    if split:
        split_excess_waits(nc)
    return nc


def _emit(nc, tc, d):
    import contextlib
    ctx = contextlib.ExitStack()
    adjq, meshTd, onehotd, maskTd = d["adjq"], d["meshTd"], d["onehotd"], d["maskTd"]
    biasd, outd = d["biasd"], d["outd"]

    meshTb = d["meshTb"]
    cpool = ctx.enter_context(tc.tile_pool(name="consts", bufs=1))
    resp = ctx.enter_context(tc.tile_pool(name="resadj", bufs=1))
    actp = ctx.enter_context(tc.tile_pool(name="acts", bufs=1))
    smallp = ctx.enter_context(tc.tile_pool(name="small", bufs=2))
    dvep = ctx.enter_context(tc.tile_pool(name="dvework", bufs=1))

    ps_misc = ctx.enter_context(tc.tile_pool(name="psmisc", bufs=2, space="PSUM"))
    ps_tp = ctx.enter_context(tc.tile_pool(name="pstp", bufs=2, space="PSUM"))
    ps_left = ctx.enter_context(tc.tile_pool(name="psleft", bufs=4, space="PSUM"))

    # ---------------- resident fp8 adjacency (sync queue) ----------------
    adjr = adjq[:].rearrange("(q p) c -> p q c", p=128)   # [128, 32, 4096]
    adjs = resp.tile([128, NQ * N], FP8, tag="adjs", name="adjs")
    adjs3 = adjs[:].rearrange("p (q c) -> p q c", c=N)
    for ib in range(NB):
        cs = slice(ib * 512, (ib + 1) * 512)
        nc.sync.dma_start(out=adjs3[:, :, cs], in_=adjr[:, :, cs])

    # ---------------- constants (gpsimd queue) ----------------
    def ctile(dram, shape, dt):
        nm = f"c_{dram.name}"
        t = cpool.tile(list(shape), dt, tag=nm, name=nm)
        nc.gpsimd.dma_start(out=t[:], in_=dram[:])
        return t

    ident = cpool.tile([128, 128], BF16)
    make_identity(nc, ident[:])
    aw1a = ctile(d["aw1ad"], [50, 128], F32)
    aw1b = ctile(d["aw1bd"], [50, 72], F32)
    aw2a = ctile(d["aw2ad"], [128, 100], F32)
    aw2b = ctile(d["aw2bd"], [72, 100], F32)
    aw3 = ctile(d["aw3d"], [100, 100], F32)
    gw0L = [ctile(d["gw0Ld"], [100, 128], F32),
            ctile(d["gw0Lpd"], [100, 128], F32)]
    selfreq = ctile(d["selfd"], [6, 62], F32)
    biases = ctile(biasd, [128, 12], F32)
    maskT = ctile(maskTd, [50, BC], F32)
    pw1 = ctile(d["pw1p"], [67, 25], BF16)
    pw2 = ctile(d["pw2d"], [25, 50], BF16)
    w3f = [ctile(d["w3fold"], [50, 128], BF16),
           ctile(d["w3foldp"], [50, 128], BF16)]
    # t4/onehot live at partitions 64:68 so the K=4 embedding matmul can run
    # row-tiled concurrently with the K=50 w3fold matmul.
    t4 = []
    for b in range(BC):
        t4t = cpool.tile([68, 128], BF16, tag=f"t4_{b}", name=f"t4_{b}")
        nc.gpsimd.dma_start(out=t4t[64:68, :], in_=d["t4d" if b == 0 else "t4pd"][:])
        t4.append(t4t)
    onehot = cpool.tile([68, BC * N], BF16, tag="onehot", name="onehot")
    nc.gpsimd.dma_start(out=onehot[64:68, :], in_=onehotd[:])
    gws = {1: [ctile(d["gw1d"], [128, 128], BF16),
               ctile(d["gw1pd"], [128, 128], BF16)],
           2: [ctile(d["gw2d"], [128, 128], BF16),
               ctile(d["gw2pd"], [128, 128], BF16)],
           3: [ctile(d["gw3d"], [128, 50], BF16),
               ctile(d["gw3pd"], [128, 50], BF16)]}

    def bcol(col, p0, p1):
        return biases[p0:p1, col:col + 1]

    # ---------------- activation tiles ----------------
    xt = actp.tile([128, BC * N], BF16, tag="x")           # [feat, b*N+n]
    fcutT = actp.tile([114, N], BF16, tag="fcutT")         # b0 rows 0:50, b1 64:114
    nc.gpsimd.memset(fcutT[:], 0.0)
    fcst = actp.tile([128, NQ * MST], FP8, tag="fcst")     # stationary slots
    nc.gpsimd.memset(fcst[:], 0.0)
    cvec = actp.tile([128, BC], F32, tag="cvec")
    cvecs = actp.tile([128, BC], F32, tag="cvecs")
    mx = actp.tile([128, NB], F32, tag="mx")
    nc.gpsimd.memset(mx[:], -1e30)
    outsb = actp.tile([128, 1], F32, tag="outsb")
    fcst3 = fcst[:].rearrange("p (q m) -> p q m", m=MST)
    # pein ring buffers: rows 30:32 and 62:64 must stay zero (pw1p has zero
    # rows there) — zeroed once here, never written in the chunk loop.
    peins = []
    for i in range(4):
        pt = actp.tile([67, 512], BF16, tag=f"pein{i}", name=f"pein{i}")
        nc.gpsimd.memset(pt[:], 0.0)
        peins.append(pt)

    # ---------------- action MLP (tiny, fp32) ----------------
    pa = ps_misc.tile([128, 2], F32, tag="misc")
    nc.tensor.matmul(pa[:], lhsT=aw1a[:], rhs=maskT[:], start=True, stop=True)
    a1a = smallp.tile([128, 2], F32, tag="a1a")
    nc.scalar.activation(a1a[:], pa[:], AF.Relu, bias=bcol(0, 0, 128))
    pb = ps_misc.tile([72, 2], F32, tag="misc")
    nc.tensor.matmul(pb[:], lhsT=aw1b[:], rhs=maskT[:], start=True, stop=True)
    a1b = smallp.tile([72, 2], F32, tag="a1b")
    nc.scalar.activation(a1b[:], pb[:], AF.Relu, bias=bcol(1, 0, 72))
    pc = ps_misc.tile([100, 2], F32, tag="misc")
    nc.tensor.matmul(pc[:], lhsT=aw2a[:], rhs=a1a[:], start=True, stop=False)
    nc.tensor.matmul(pc[:], lhsT=aw2b[:], rhs=a1b[:], start=False, stop=True)
    a2 = smallp.tile([100, 2], F32, tag="a2")
    nc.scalar.activation(a2[:], pc[:], AF.Relu, bias=bcol(2, 0, 100))
    pd = ps_misc.tile([100, 2], F32, tag="misc")
    nc.tensor.matmul(pd[:], lhsT=aw3[:], rhs=a2[:], start=True, stop=True)
    a3 = smallp.tile([100, 2], F32, tag="a3")
    nc.scalar.activation(a3[:], pd[:], AF.Identity, bias=bcol(3, 0, 100))
    pe_ = ps_misc.tile([128, 2], F32, tag="misc")
    for b in range(BC):
        nc.tensor.matmul(pe_[:, b:b + 1], lhsT=gw0L[b][:], rhs=a3[:, b:b + 1],
                         start=True, stop=True)
        nc.scalar.activation(cvec[:, b:b + 1], pe_[:, b:b + 1], AF.Identity,
                             bias=bcol(4 + b, 0, 128))
    nc.scalar.activation(cvecs[:], cvec[:], AF.Identity, scale=SF)

    # ---------------- positional front-end + fused layer-0 features -------
    # batch-1 cut rows live at partitions 64:106 via the host-side column
    # permutation of w3foldp/t4pd; right-part rows land at 0:64 and 106:128.
    for ch in range(NB):
        cs = slice(ch * 512, (ch + 1) * 512)
        m6 = smallp.tile([6, 512], F32, tag="m6")
        nc.scalar.dma_start(
            out=m6[:],
            in_=meshTd[:, :, cs].rearrange("b c n -> (b c) n"))
        # t[30b+k, n] = mesh[b, k%3, n] * freq[k//3] / (2*pi)
        t2 = ps_misc.tile([62, 512], F32, tag="misc")
        nc.tensor.matmul(t2[:], lhsT=selfreq[:], rhs=m6[:], start=True, stop=True)
        # range reduction: d = t - round(t); dc = (t+0.25) - round(t+0.25)
        # (all on DVE: gpsimd is ~14x slower for bulk elementwise work)
        r1 = dvep.tile([62, 512], F32, tag="r1")
        nc.vector.tensor_scalar_add(r1[:], t2[:], MAGIC)
        r2 = dvep.tile([62, 512], F32, tag="r2")
        nc.vector.tensor_scalar_add(r2[:], r1[:], -MAGIC)
        dd = dvep.tile([62, 512], F32, tag="dd")
        nc.vector.tensor_sub(dd[:], t2[:], r2[:])
        tcq = dvep.tile([62, 512], F32, tag="tcq")
        nc.vector.tensor_scalar_add(tcq[:], t2[:], 0.25)
        nc.vector.tensor_scalar_add(r1[:], tcq[:], MAGIC)
        nc.vector.tensor_scalar_add(r2[:], r1[:], -MAGIC)
        dc = dvep.tile([62, 512], F32, tag="dc")
        nc.vector.tensor_sub(dc[:], tcq[:], r2[:])
        for b in range(BC):
            xs = slice(b * N + ch * 512, b * N + (ch + 1) * 512)
            pein = peins[2 * (ch % 2) + b]
            nc.scalar.activation(pein[0:30, :], dd[32 * b:32 * b + 30, :],
                                 AF.Sin, scale=TWO_PI)
            nc.scalar.activation(pein[32:62, :], dc[32 * b:32 * b + 30, :],
                                 AF.Sin, scale=TWO_PI)
            nc.scalar.dma_start(out=pein[64:67, :], in_=meshTb[b, :, cs])
            # h1 = relu(pe_in @ pw1 + pb1)
            ph1 = ps_tp.tile([25, 512], F32, tag="tp")
            nc.tensor.matmul(ph1[:], lhsT=pw1[:], rhs=pein[:],
                             start=True, stop=True)
            h1 = smallp.tile([25, 512], BF16, tag=f"h1{b}")
            nc.scalar.activation(h1[:], ph1[:], AF.Relu, bias=bcol(6, 0, 25))
            # h2 = relu(h1 @ pw2 + pb2)
            ph2 = ps_tp.tile([50, 512], F32, tag="tp")
            nc.tensor.matmul(ph2[:], lhsT=pw2[:], rhs=h1[:],
                             start=True, stop=True)
            h2 = smallp.tile([50, 512], BF16, tag=f"h2{b}")
            nc.vector.tensor_scalar(h2[:], ph2[:], bcol(7, 0, 50), 0.0,
                                    mybir.AluOpType.add, mybir.AluOpType.max)
            # layer-0 features: f0 = [h2 | onehot] @ [w3fold; t4] (+ cvec);
            # the K=50 and K=4 matmuls run row-tiled concurrently.
            pf = ps_misc.tile([128, 512], F32, tag="misc")
            nc.tensor.matmul(pf[:], lhsT=w3f[b][:], rhs=h2[:],
                             start=True, stop=False)
            nc.tensor.matmul(pf[:], lhsT=t4[b][64:68, :],
                             rhs=onehot[64:68, xs],
                             start=False, stop=True)
            # full-tile relu (DVE): rows overlapping the cut range get
            # garbage and are overwritten by the C drain later.
            nc.vector.tensor_scalar(xt[:, xs], pf[:, :],
                                    cvec[:, b:b + 1], 0.0,
                                    mybir.AluOpType.add, mybir.AluOpType.max)
            if b == 0:
                nc.scalar.activation(fcutT[0:42, cs], pf[0:42, :],
                                     AF.Identity, bias=cvecs[0:42, 0:1],
                                     scale=SF)
            else:
                nc.scalar.activation(fcutT[64:106, cs], pf[64:106, :],
                                     AF.Identity, bias=cvecs[64:106, 1:2],
                                     scale=SF)

    # ---------------- GCN layers ----------------
    for li in range(4):
        last = li == 3
        cd = 50 if last else 42
        mm = 64 + cd                       # stationary packed width
        # phase A: f = x @ gw (skipped for li=0: fused above)
        if li > 0:
            for ch in range(NB):
                if last:
                    pf = ps_misc.tile([128, 512], F32, tag="misc")
                    for b in range(BC):
                        xs = slice(b * N + ch * 512, b * N + (ch + 1) * 512)
                        nc.tensor.matmul(pf[64 * b:64 * b + 50, :],
                                         lhsT=gws[3][b][:], rhs=xt[:, xs],
                                         start=True, stop=True)
                    cs = slice(ch * 512, (ch + 1) * 512)
                    nc.scalar.activation(fcutT[0:50, cs], pf[0:50, :],
                                         AF.Identity, scale=SF)
                    nc.scalar.activation(fcutT[64:114, cs], pf[64:114, :],
                                         AF.Identity, scale=SF)
                else:
                    for b in range(BC):
                        xs = slice(b * N + ch * 512, b * N + (ch + 1) * 512)
                        cs = slice(ch * 512, (ch + 1) * 512)
                        pf = ps_misc.tile([128, 512], F32, tag="misc")
                        nc.tensor.matmul(pf[:], lhsT=gws[li][b][:],
                                         rhs=xt[:, xs], start=True, stop=True)
                        # full-tile relu; cut rows get garbage and are
                        # rewritten by the C drain.
                        if b == 0:
                            nc.scalar.activation(xt[:, xs], pf[:, :], AF.Relu)
                        else:
                            nc.vector.tensor_scalar_max(xt[:, xs], pf[:, :],
                                                        0.0)
                        if b == 0:
                            nc.scalar.activation(fcutT[0:42, cs], pf[0:42, :],
                                                 AF.Identity, scale=SF)
                        else:
                            nc.scalar.activation(fcutT[64:106, cs],
                                                 pf[64:106, :],
                                                 AF.Identity, scale=SF)
        # phase B: one transpose per 128-node block serves both batches
        for q in range(NQ):
            jc = slice(q * 128, (q + 1) * 128)
            tp = ps_tp.tile([128, 128], BF16, tag="tp")
            nc.tensor.transpose(tp[:, 0:mm], fcutT[0:mm, jc],
                                ident[0:mm, 0:mm])
            if q % 2 == 0:
                nc.vector.tensor_copy(fcst3[:, q, 0:mm], tp[:, 0:mm])
            else:
                nc.scalar.activation(fcst3[:, q, 0:mm], tp[:, 0:mm],
                                     AF.Identity)
        # phase C: left^T = fcst.T @ adjT in fp8 DoubleRow, jt pairs of q.
        # li=0 uses pairs so compute can start before the whole adjacency
        # has landed in SBUF; later layers use groups of 4 (fewer PSUM swaps).
        if li == 0:
            groups = [(0, 2), (2, 4), (4, 6), (6, 8)]
        else:
            groups = [(0, 4), (4, 8)]
        for g0, g1 in groups:
            pls = {}
            for ib in range(g0, g1):
                pls[ib] = ps_left.tile([mm, 512], F32, tag="left",
                                       name=f"pl{li}_{ib}")
            for jt in range(NQ // 2):
                lhsT = fcst3[:, 2 * jt:2 * jt + 2, 0:mm]
                for ib in range(g0, g1):
                    rhs = adjs3[:, 2 * jt:2 * jt + 2,
                                ib * 512:(ib + 1) * 512]
                    nc.tensor.matmul(pls[ib][:], lhsT=lhsT, rhs=rhs,
                                     start=(jt == 0), stop=(jt == NQ // 2 - 1),
                                     perf_mode=DR)
            for ib in range(g0, g1):
                pl = pls[ib]
                if not last:
                    # xt keeps the (SA*SF)-scaled cut values; the next-layer
                    # gw cut rows are pre-divided on the host, and the gb cut
                    # biases are pre-multiplied (cols 8-10).
                    nc.vector.tensor_scalar(
                        xt[0:42, ib * 512:(ib + 1) * 512],
                        pl[0:42, :], bcol(8 + li, 0, 42), 0.0,
                        mybir.AluOpType.add, mybir.AluOpType.max)
                    nc.vector.tensor_scalar(
                        xt[64:106, N + ib * 512:N + (ib + 1) * 512],
                        pl[64:106, :], bcol(8 + li, 64, 106), 0.0,
                        mybir.AluOpType.add, mybir.AluOpType.max)
                else:
                    nc.vector.tensor_reduce(
                        mx[0:50, ib:ib + 1], pl[0:50, :],
                        mybir.AxisListType.X, mybir.AluOpType.max)
                    nc.vector.tensor_reduce(
                        mx[64:114, ib:ib + 1], pl[64:114, :],
                        mybir.AxisListType.X, mybir.AluOpType.max)

    # ---------------- final max + bias + output ----------------
    mxr = smallp.tile([128, 1], F32, tag="mxr")
    nc.vector.tensor_reduce(mxr[:], mx[:], mybir.AxisListType.X,
                            mybir.AluOpType.max)
    nc.scalar.activation(outsb[:], mxr[:], AF.Identity, bias=bcol(11, 0, 128),
                         scale=INV)
    nc.sync.dma_start(out=outd[:], in_=outsb[:])
    ctx.close()


# ---------------------------------------------------------------------------
# host side
# ---------------------------------------------------------------------------

# batch-1 feature permutation: rows 0:64 <- features 42:106,
# rows 64:106 <- features 0:42 (the cut), rows 106:128 <- features 106:128
PI = np.concatenate([np.arange(42, 106), np.arange(0, 42),
                     np.arange(106, 128)]).astype(np.int64)


def _prep_shared(inp):
    """Host preprocessing shared across cores (weights + adj)."""
    f32 = np.float32
    adjq = np.ascontiguousarray(
        inp["adj"].astype(f32).T * f32(SA)).astype(E4)

    gw0 = inp["gw0"].astype(f32)
    w3fold = (inp["pw3"].astype(f32) @ gw0[100:200])
    t4 = (inp["emb"].astype(f32) @ gw0[200:300])
    pb3f = (inp["pb3"].astype(f32) @ gw0[100:200]).astype(f32)
    gw0L = np.ascontiguousarray(gw0[:100])

    # pe_in row permutation: ours = [sin(f,c) x30 | cos(f,c) x30 | mesh x3]
    pw1f = inp["pw1"].astype(f32)
    pw1p_ = np.zeros((67, 25), f32)
    for k in range(30):
        f, c = divmod(k, 3)
        pw1p_[k] = pw1f[f * 6 + c]          # sin rows
        pw1p_[32 + k] = pw1f[f * 6 + 3 + c]  # cos rows
    pw1p_[64:67] = pw1f[60:63]
    pw1p = pw1p_.astype(BF)

    freqs = np.asarray([np.pi] + [2.0 * np.pi * i for i in range(1, 10)], f32)
    freq2 = np.repeat(freqs, 3) / (2.0 * np.pi)   # [30]
    self6 = np.zeros((6, 62), f32)
    for b in range(2):
        for k in range(30):
            self6[3 * b + k % 3, 32 * b + k] = freq2[k]

    # xt carries the adjacency-matmul output still scaled by SA*SF; undo the
    # scale by pre-dividing the gw rows that consume cut features (b0 rows
    # 0:42, permuted-b1 rows 64:106) and pre-multiplying the gb cut biases.
    gw1 = inp["gw1"].astype(f32)
    gw2 = inp["gw2"].astype(f32)
    gw3 = inp["gw3"].astype(f32)
    gw1p = gw1[np.ix_(PI, PI)].copy()
    gw2p = gw2[np.ix_(PI, PI)].copy()
    gw3p = gw3[PI, :].copy()
    for g in (gw1, gw2, gw3):
        g[0:42] *= f32(INV)
    for g in (gw1p, gw2p, gw3p):
        g[64:106] *= f32(INV)

    biasd = np.zeros((128, 12), f32)
    biasd[0:128, 0] = inp["ab1"][:128]
    biasd[0:72, 1] = inp["ab1"][128:200]
    biasd[0:100, 2] = inp["ab2"]
    biasd[0:100, 3] = inp["ab3"]
    biasd[0:128, 4] = pb3f
    biasd[0:128, 5] = pb3f[PI]
    biasd[0:25, 6] = inp["pb1"].astype(f32)
    biasd[0:50, 7] = inp["pb2"].astype(f32)
    for li in range(3):
        gb = inp[f"gb{li}"].astype(f32) * f32(SA * SF)
        biasd[0:42, 8 + li] = gb[:42]
        biasd[64:106, 8 + li] = gb[:42]
    gb3 = inp["gb3"].astype(f32)
    biasd[0:50, 11] = gb3
    biasd[64:114, 11] = gb3

    return {
        "adjq": adjq,
        "pw1p": pw1p,
        "pw2d": inp["pw2"].astype(BF),
        "w3fold": w3fold.astype(BF),
        "w3foldp": np.ascontiguousarray(w3fold[:, PI]).astype(BF),
        "t4d": t4.astype(BF),
        "t4pd": np.ascontiguousarray(t4[:, PI]).astype(BF),
        "gw1d": gw1.astype(BF),
        "gw1pd": np.ascontiguousarray(gw1p).astype(BF),
        "gw2d": gw2.astype(BF),
        "gw2pd": np.ascontiguousarray(gw2p).astype(BF),
        "gw3d": gw3.astype(BF),
        "gw3pd": np.ascontiguousarray(gw3p).astype(BF),
        "aw1ad": np.ascontiguousarray(inp["aw1"].astype(f32)[:, :128]),
        "aw1bd": np.ascontiguousarray(inp["aw1"].astype(f32)[:, 128:200]),
        "aw2ad": np.ascontiguousarray(inp["aw2"].astype(f32)[:128]),
        "aw2bd": np.ascontiguousarray(inp["aw2"].astype(f32)[128:200]),
        "aw3d": inp["aw3"].astype(f32),
        "gw0Ld": gw0L,
        "gw0Lpd": np.ascontiguousarray(gw0L[:, PI]),
        "selfd": self6,
        "biasd": biasd,
    }


def _prep_core(inp, shared, core):
    bs = slice(core * BC, (core + 1) * BC)
    f32 = np.float32
    mesh = inp["mesh"].astype(f32)[bs]                       # [2, N, 3]
    meshT = np.ascontiguousarray(mesh.transpose(0, 2, 1))    # [2, 3, N]
    mi = inp["mask_idx"][bs]                                 # [2, N] int32
    onehot = (mi[:, None, :] == np.arange(4, dtype=mi.dtype)[None, :, None])
    onehot = np.ascontiguousarray(
        onehot.transpose(1, 0, 2).reshape(4, BC * N)).astype(BF)
    maskT = np.ascontiguousarray(inp["mask"].astype(f32)[bs].T)  # [50, 2]
    m = dict(shared)
    m["meshTd"] = meshT
    m["meshTb"] = meshT.astype(BF)
    m["onehotd"] = onehot
    m["maskTd"] = maskT
    return m


_CACHED = {}


def kernel(**inputs) -> np.ndarray:
    if "nc" not in _CACHED:
        _CACHED["nc"] = build_bass()
    nc = _CACHED["nc"]
    shared = _prep_shared(inputs)
    in_maps = [_prep_core(inputs, shared, c) for c in range(NCORES)]
    res = run_bass_kernel_spmd(nc, in_maps, list(range(NCORES)), **run_kwargs)
    out = np.empty((B, 50), np.float32)
    for c in range(NCORES):
        o = res.results[c]["outd"][:, 0]
        out[2 * c] = o[0:50]
        out[2 * c + 1] = o[64:114]
    _CACHED["last_results"] = res
    return out


# revision 39
# speedup vs baseline: 1.5821x; 1.0540x over previous
"""Trainium2 Bass kernel for the GNN message-passing model.

Strategy: pure data-parallel over batch (B=16 -> 2 batches per core, 8 cores,
no cross-core communication).

Key design points vs the earlier baseline:
  * The whole adjacency (transposed) is kept RESIDENT in SBUF as fp8-e4m3
    (16 MiB), scaled by 2^19 so values land in e4m3's normal range. This
    removes the per-layer HBM re-streaming (~96 MiB/core) that caused DMA
    waits and HAM clock-throttle oscillation.
  * The adjacency matmuls (the dominant cost) run in fp8 DoubleRow perf mode:
    contraction pairs are packed 2-per-cell, halving the number of
    accumulation passes (16 instead of 32 per 512-col output tile).
  * Batch 1's feature order is PERMUTED (cut features moved to partition
    rows 64:106/114) so both batches' cut features live in one SBUF tile and
    a single PE transpose per 128-node block serves both batches
    (32 transposes/layer instead of 64).
  * Layer-0 feature matmul is fused into the positional front-end chunk loop
    so h1/h2 activations never need full-length SBUF tiles.

Weight-only folds done on host (pure parameter preprocessing):
  W3fold = pw3 @ gw0[100:200]   (positional-MLP last layer folded into gw0)
  t4     = emb @ gw0[200:300]   (embedding table folded into gw0)
  pb3f   = pb3 @ gw0[100:200]   (bias fold)
mask_idx is re-encoded as a one-hot (4 classes) so the embedding lookup
becomes a K=4 matmul accumulated into the same PSUM as the layer-0 matmul.
"""

import numpy as np
import ml_dtypes

import concourse.bass as bass
import concourse.mybir as mybir
import concourse.tile as tile
from concourse.masks import make_identity
from concourse.bass_utils import run_bass_kernel_spmd

F32 = mybir.dt.float32
BF16 = mybir.dt.bfloat16
FP8 = mybir.dt.float8e4
AF = mybir.ActivationFunctionType
BF = ml_dtypes.bfloat16
E4 = ml_dtypes.float8_e4m3

B, N, BC = 16, 4096, 2          # batches, nodes, batches per core
NCORES = 8
NB = N // 512                   # 8 column blocks of 512
NQ = N // 128                   # 32 contraction sub-blocks of 128
MAGIC = float(1.5 * 2 ** 23)    # fp32 round-to-nearest magic constant
TWO_PI = float(2.0 * np.pi)
SA = float(2.0 ** 19)           # adjacency fp8 scale
SF = 64.0                       # cut-feature fp8 scale
INV = float(1.0 / (SA * SF))    # undo scale after the adjacency matmul
MST = 128                       # fcst stationary slot stride (bytes, fp8)
DR = mybir.MatmulPerfMode.DoubleRow

run_kwargs = {}                 # test.py may inject trace kwargs here


def split_excess_waits(nc, max_waits=1):
    """Walrus codegen on this image rejects >1 sem wait per instruction;
    move excess waits onto preceding same-engine no-ops."""
    n_split = 0
    for fn in nc.m.functions:
        for blk in fn.blocks:
            insts = list(blk.instructions)
            out = []
            changed = False
            for inst in insts:
                si = getattr(inst, "sync_info", None)
                if si is not None and len(si.on_wait) > max_waits:
                    waits = list(si.on_wait)
                    chunks = [waits[i:i + max_waits]
                              for i in range(0, len(waits), max_waits)]
                    for ci, ch in enumerate(chunks[:-1]):
                        nop = mybir.InstNoOp(
                            name=f"{inst.name}-wsplit-{ci}", ins=[], outs=[])
                        nop.engine = inst.engine
                        nop.sync_info = mybir.SyncInfo(on_wait=ch, on_update=[])
                        out.append(nop)
                        n_split += 1
                    inst.sync_info = mybir.SyncInfo(
                        on_wait=chunks[-1], on_update=list(si.on_update))
                    changed = True
                out.append(inst)
            if changed:
                blk.instructions = out
    return n_split


def _param(nc, name, shape, dt):
    return nc.declare_dram_parameter(name, list(shape), dt, isOutput=False)


def build_bass(split=True):
    nc = bass.Bass()

    adjq = _param(nc, "adjq", [N, N], FP8)
    meshTd = _param(nc, "meshTd", [BC, 3, N], F32)
    meshTb = _param(nc, "meshTb", [BC, 3, N], BF16)
    onehotd = _param(nc, "onehotd", [4, BC * N], BF16)
    maskTd = _param(nc, "maskTd", [50, BC], F32)

    pw1p = _param(nc, "pw1p", [67, 25], BF16)
    pw2d = _param(nc, "pw2d", [25, 50], BF16)
    w3fold = _param(nc, "w3fold", [50, 128], BF16)
    w3foldp = _param(nc, "w3foldp", [50, 128], BF16)
    t4d = _param(nc, "t4d", [4, 128], BF16)
    t4pd = _param(nc, "t4pd", [4, 128], BF16)
    gw1d = _param(nc, "gw1d", [128, 128], BF16)
    gw1pd = _param(nc, "gw1pd", [128, 128], BF16)
    gw2d = _param(nc, "gw2d", [128, 128], BF16)
    gw2pd = _param(nc, "gw2pd", [128, 128], BF16)
    gw3d = _param(nc, "gw3d", [128, 50], BF16)
    gw3pd = _param(nc, "gw3pd", [128, 50], BF16)
    aw1ad = _param(nc, "aw1ad", [50, 128], F32)
    aw1bd = _param(nc, "aw1bd", [50, 72], F32)
    aw2ad = _param(nc, "aw2ad", [128, 100], F32)
    aw2bd = _param(nc, "aw2bd", [72, 100], F32)
    aw3d = _param(nc, "aw3d", [100, 100], F32)
    gw0Ld = _param(nc, "gw0Ld", [100, 128], F32)
    gw0Lpd = _param(nc, "gw0Lpd", [100, 128], F32)
    selfd = _param(nc, "selfd", [6, 62], F32)
    biasd = _param(nc, "biasd", [128, 12], F32)
    # bias columns: 0 ab1a[128], 1 ab1b[72], 2 ab2[100], 3 ab3[100],
    # 4 pb3f[128], 5 pb3f_perm[128], 6 pb1[25], 7 pb2[50],
    # 8/9/10 gb{0,1,2}cut (rows 0:42 + 64:106), 11 gb3 (rows 0:50 + 64:114)
    outd = nc.declare_dram_parameter("outd", [128, 1], F32, isOutput=True)

    with tile.TileContext(nc) as tc:
        _emit(nc, tc, locals())
    if split:
        split_excess_waits(nc)
    return nc


def _emit(nc, tc, d):
    import contextlib
    ctx = contextlib.ExitStack()
    adjq, meshTd, onehotd, maskTd = d["adjq"], d["meshTd"], d["onehotd"], d["maskTd"]
    biasd, outd = d["biasd"], d["outd"]

    meshTb = d["meshTb"]
    cpool = ctx.enter_context(tc.tile_pool(name="consts", bufs=1))
    resp = ctx.enter_context(tc.tile_pool(name="resadj", bufs=1))
    actp = ctx.enter_context(tc.tile_pool(name="acts", bufs=1))
    smallp = ctx.enter_context(tc.tile_pool(name="small", bufs=2))
    dvep = ctx.enter_context(tc.tile_pool(name="dvework", bufs=1))

    ps_misc = ctx.enter_context(tc.tile_pool(name="psmisc", bufs=2, space="PSUM"))
    ps_tp = ctx.enter_context(tc.tile_pool(name="pstp", bufs=2, space="PSUM"))
    ps_left = ctx.enter_context(tc.tile_pool(name="psleft", bufs=4, space="PSUM"))

    # ---------------- constants + adjacency DMA scheduling ----------------
    # action-MLP weights go on the scalar queue (free early) so the action
    # MLP starts immediately; front-end consts lead the gpsimd queue; the
    # adjacency splits between sync (ib 0-3) and gpsimd (ib 4-7).
    def ctile(dram, shape, dt, eng=None):
        nm = f"c_{dram.name}"
        t = cpool.tile(list(shape), dt, tag=nm, name=nm)
        (eng or nc.gpsimd).dma_start(out=t[:], in_=dram[:])
        return t

    aw1a = ctile(d["aw1ad"], [50, 128], F32, nc.scalar)
    aw1b = ctile(d["aw1bd"], [50, 72], F32, nc.scalar)
    aw2a = ctile(d["aw2ad"], [128, 100], F32, nc.scalar)
    aw2b = ctile(d["aw2bd"], [72, 100], F32, nc.scalar)
    aw3 = ctile(d["aw3d"], [100, 100], F32, nc.scalar)
    gw0L = [ctile(d["gw0Ld"], [100, 128], F32, nc.scalar),
            ctile(d["gw0Lpd"], [100, 128], F32, nc.scalar)]
    biases = ctile(biasd, [128, 12], F32, nc.scalar)
    maskT = ctile(maskTd, [50, BC], F32, nc.scalar)

    adjr = adjq[:].rearrange("(q p) c -> p q c", p=128)   # [128, 32, 4096]
    adjs = resp.tile([128, NQ * N], FP8, tag="adjs", name="adjs")
    adjs3 = adjs[:].rearrange("p (q c) -> p q c", c=N)
    for ib in range(4):
        cs = slice(ib * 512, (ib + 1) * 512)
        nc.sync.dma_start(out=adjs3[:, :, cs], in_=adjr[:, :, cs])

    selfreq = ctile(d["selfd"], [6, 62], F32)
    pw1 = ctile(d["pw1p"], [67, 25], BF16)
    pw2 = ctile(d["pw2d"], [25, 50], BF16)
    w3f = [ctile(d["w3fold"], [50, 128], BF16),
           ctile(d["w3foldp"], [50, 128], BF16)]
    # t4/onehot live at partitions 64:68 so the K=4 embedding matmul can run
    # row-tiled concurrently with the K=50 w3fold matmul.
    t4 = []
    for b in range(BC):
        t4t = cpool.tile([68, 128], BF16, tag=f"t4_{b}", name=f"t4_{b}")
        nc.gpsimd.dma_start(out=t4t[64:68, :], in_=d["t4d" if b == 0 else "t4pd"][:])
        t4.append(t4t)
    onehot = cpool.tile([68, BC * N], BF16, tag="onehot", name="onehot")
    nc.gpsimd.dma_start(out=onehot[64:68, :], in_=onehotd[:])
    ident = cpool.tile([128, 128], BF16)
    make_identity(nc, ident[:])

    def bcol(col, p0, p1):
        return biases[p0:p1, col:col + 1]

    # ---------------- activation tiles ----------------
    xt = actp.tile([128, BC * N], BF16, tag="x")           # [feat, b*N+n]
    fcutT = actp.tile([114, N], BF16, tag="fcutT")         # b0 rows 0:50, b1 64:114
    nc.gpsimd.memset(fcutT[:], 0.0)
    fcst = actp.tile([128, NQ * MST], FP8, tag="fcst")     # stationary slots
    nc.gpsimd.memset(fcst[:], 0.0)
    cvec = actp.tile([128, BC], F32, tag="cvec")
    cvecs = actp.tile([128, BC], F32, tag="cvecs")
    mx = actp.tile([128, NB], F32, tag="mx")
    nc.gpsimd.memset(mx[:], -1e30)
    outsb = actp.tile([128, 1], F32, tag="outsb")
    fcst3 = fcst[:].rearrange("p (q m) -> p q m", m=MST)
    # pein ring buffers: rows 30:32 and 62:64 must stay zero (pw1p has zero
    # rows there) — zeroed once here (DVE), never written in the chunk loop.
    peins = []
    for i in range(4):
        pt = actp.tile([67, 512], BF16, tag=f"pein{i}", name=f"pein{i}")
        nc.vector.memset(pt[:], 0.0)
        peins.append(pt)
    # adjacency tail blocks + later-layer weights follow on the gpsimd queue
    for ib in range(4, NB):
        cs = slice(ib * 512, (ib + 1) * 512)
        nc.gpsimd.dma_start(out=adjs3[:, :, cs], in_=adjr[:, :, cs])
    gws = {1: [ctile(d["gw1d"], [128, 128], BF16),
               ctile(d["gw1pd"], [128, 128], BF16)],
           2: [ctile(d["gw2d"], [128, 128], BF16),
               ctile(d["gw2pd"], [128, 128], BF16)],
           3: [ctile(d["gw3d"], [128, 50], BF16),
               ctile(d["gw3pd"], [128, 50], BF16)]}

    # ---------------- action MLP (tiny, fp32) ----------------
    pa = ps_misc.tile([128, 2], F32, tag="misc")
    nc.tensor.matmul(pa[:], lhsT=aw1a[:], rhs=maskT[:], start=True, stop=True)
    a1a = smallp.tile([128, 2], F32, tag="a1a")
    nc.scalar.activation(a1a[:], pa[:], AF.Relu, bias=bcol(0, 0, 128))
    pb = ps_misc.tile([72, 2], F32, tag="misc")
    nc.tensor.matmul(pb[:], lhsT=aw1b[:], rhs=maskT[:], start=True, stop=True)
    a1b = smallp.tile([72, 2], F32, tag="a1b")
    nc.scalar.activation(a1b[:], pb[:], AF.Relu, bias=bcol(1, 0, 72))
    pc = ps_misc.tile([100, 2], F32, tag="misc")
    nc.tensor.matmul(pc[:], lhsT=aw2a[:], rhs=a1a[:], start=True, stop=False)
    nc.tensor.matmul(pc[:], lhsT=aw2b[:], rhs=a1b[:], start=False, stop=True)
    a2 = smallp.tile([100, 2], F32, tag="a2")
    nc.scalar.activation(a2[:], pc[:], AF.Relu, bias=bcol(2, 0, 100))
    pd = ps_misc.tile([100, 2], F32, tag="misc")
    nc.tensor.matmul(pd[:], lhsT=aw3[:], rhs=a2[:], start=True, stop=True)
    a3 = smallp.tile([100, 2], F32, tag="a3")
    nc.scalar.activation(a3[:], pd[:], AF.Identity, bias=bcol(3, 0, 100))
    pe_ = ps_misc.tile([128, 2], F32, tag="misc")
    for b in range(BC):
        nc.tensor.matmul(pe_[:, b:b + 1], lhsT=gw0L[b][:], rhs=a3[:, b:b + 1],
                         start=True, stop=True)
        nc.scalar.activation(cvec[:, b:b + 1], pe_[:, b:b + 1], AF.Identity,
                             bias=bcol(4 + b, 0, 128))
    nc.scalar.activation(cvecs[:], cvec[:], AF.Identity, scale=SF)

    # one transpose per 128-node block serves both batches; the PSUM->SBUF
    # copies alternate between DVE and scalar.
    def emit_tp(q, mm):
        jc = slice(q * 128, (q + 1) * 128)
        tp = ps_tp.tile([128, 128], BF16, tag="tp", name=f"tp_{q}")
        nc.tensor.transpose(tp[:, 0:mm], fcutT[0:mm, jc], ident[0:mm, 0:mm])
        if q % 2 == 0:
            nc.vector.tensor_copy(fcst3[:, q, 0:mm], tp[:, 0:mm])
        else:
            nc.scalar.activation(fcst3[:, q, 0:mm], tp[:, 0:mm], AF.Identity)

    # ---------------- positional front-end + fused layer-0 features -------
    # batch-1 cut rows live at partitions 64:106 via the host-side column
    # permutation of w3foldp/t4pd; right-part rows land at 0:64 and 106:128.
    # Layer-0 transposes are woven in one chunk behind the A drains.
    for ch in range(NB):
        cs = slice(ch * 512, (ch + 1) * 512)
        m6 = smallp.tile([6, 512], F32, tag="m6")
        nc.scalar.dma_start(
            out=m6[:],
            in_=meshTd[:, :, cs].rearrange("b c n -> (b c) n"))
        # t[30b+k, n] = mesh[b, k%3, n] * freq[k//3] / (2*pi)
        t2 = ps_misc.tile([62, 512], F32, tag="misc")
        nc.tensor.matmul(t2[:], lhsT=selfreq[:], rhs=m6[:], start=True, stop=True)
        # range reduction: d = t - round(t); dc = (t+0.25) - round(t+0.25)
        # (all on DVE: gpsimd is ~14x slower for bulk elementwise work)
        r1 = dvep.tile([62, 512], F32, tag="r1")
        nc.vector.tensor_scalar_add(r1[:], t2[:], MAGIC)
        r2 = dvep.tile([62, 512], F32, tag="r2")
        nc.vector.tensor_scalar_add(r2[:], r1[:], -MAGIC)
        dd = dvep.tile([62, 512], F32, tag="dd")
        nc.vector.tensor_sub(dd[:], t2[:], r2[:])
        tcq = dvep.tile([62, 512], F32, tag="tcq")
        nc.vector.tensor_scalar_add(tcq[:], t2[:], 0.25)
        nc.vector.tensor_scalar_add(r1[:], tcq[:], MAGIC)
        nc.vector.tensor_scalar_add(r2[:], r1[:], -MAGIC)
        dc = dvep.tile([62, 512], F32, tag="dc")
        nc.vector.tensor_sub(dc[:], tcq[:], r2[:])
        for b in range(BC):
            xs = slice(b * N + ch * 512, b * N + (ch + 1) * 512)
            pein = peins[2 * (ch % 2) + b]
            nc.scalar.activation(pein[0:30, :], dd[32 * b:32 * b + 30, :],
                                 AF.Sin, scale=TWO_PI)
            nc.scalar.activation(pein[32:62, :], dc[32 * b:32 * b + 30, :],
                                 AF.Sin, scale=TWO_PI)
            nc.scalar.dma_start(out=pein[64:67, :], in_=meshTb[b, :, cs])
            # h1 = relu(pe_in @ pw1 + pb1)
            ph1 = ps_tp.tile([25, 512], F32, tag="tp")
            nc.tensor.matmul(ph1[:], lhsT=pw1[:], rhs=pein[:],
                             start=True, stop=True)
            h1 = smallp.tile([25, 512], BF16, tag=f"h1{b}")
            nc.scalar.activation(h1[:], ph1[:], AF.Relu, bias=bcol(6, 0, 25))
            # h2 = relu(h1 @ pw2 + pb2)
            ph2 = ps_tp.tile([50, 512], F32, tag="tp")
            nc.tensor.matmul(ph2[:], lhsT=pw2[:], rhs=h1[:],
                             start=True, stop=True)
            h2 = smallp.tile([50, 512], BF16, tag=f"h2{b}")
            nc.vector.tensor_scalar(h2[:], ph2[:], bcol(7, 0, 50), 0.0,
                                    mybir.AluOpType.add, mybir.AluOpType.max)
            # layer-0 features: f0 = [h2 | onehot] @ [w3fold; t4] (+ cvec);
            # the K=50 and K=4 matmuls run row-tiled concurrently.
            pf = ps_misc.tile([128, 512], F32, tag="misc")
            nc.tensor.matmul(pf[:], lhsT=w3f[b][:], rhs=h2[:],
                             start=True, stop=False)
            nc.tensor.matmul(pf[:], lhsT=t4[b][64:68, :],
                             rhs=onehot[64:68, xs],
                             start=False, stop=True)
            # full-tile relu (DVE): rows overlapping the cut range get
            # garbage and are overwritten by the C drain later.
            nc.vector.tensor_scalar(xt[:, xs], pf[:, :],
                                    cvec[:, b:b + 1], 0.0,
                                    mybir.AluOpType.add, mybir.AluOpType.max)
            if b == 0:
                nc.scalar.activation(fcutT[0:42, cs], pf[0:42, :],
                                     AF.Identity, bias=cvecs[0:42, 0:1],
                                     scale=SF)
            else:
                nc.scalar.activation(fcutT[64:106, cs], pf[64:106, :],
                                     AF.Identity, bias=cvecs[64:106, 1:2],
                                     scale=SF)
        if ch >= 1:
            for q in range(4 * (ch - 1), 4 * ch):
                emit_tp(q, 106)
    for q in range(4 * (NB - 1), NQ):
        emit_tp(q, 106)

    # ---------------- GCN layers ----------------
    for li in range(4):
        last = li == 3
        cd = 50 if last else 42
        mm = 64 + cd                       # stationary packed width
        # phase A: f = x @ gw (skipped for li=0: fused above), with the
        # transposes woven in one chunk behind the drains
        if li > 0:
            for ch in range(NB):
                if ch >= 1:
                    for q in range(4 * (ch - 1), 4 * ch):
                        emit_tp(q, mm)
                if last:
                    pf = ps_misc.tile([128, 512], F32, tag="misc")
                    for b in range(BC):
                        xs = slice(b * N + ch * 512, b * N + (ch + 1) * 512)
                        nc.tensor.matmul(pf[64 * b:64 * b + 50, :],
                                         lhsT=gws[3][b][:], rhs=xt[:, xs],
                                         start=True, stop=True)
                    cs = slice(ch * 512, (ch + 1) * 512)
                    nc.scalar.activation(fcutT[0:50, cs], pf[0:50, :],
                                         AF.Identity, scale=SF)
                    nc.scalar.activation(fcutT[64:114, cs], pf[64:114, :],
                                         AF.Identity, scale=SF)
                else:
                    for b in range(BC):
                        xs = slice(b * N + ch * 512, b * N + (ch + 1) * 512)
                        cs = slice(ch * 512, (ch + 1) * 512)
                        pf = ps_misc.tile([128, 512], F32, tag="misc")
                        nc.tensor.matmul(pf[:], lhsT=gws[li][b][:],
                                         rhs=xt[:, xs], start=True, stop=True)
                        # full-tile relu; cut rows get garbage and are
                        # rewritten by the C drain.
                        if b == 0:
                            nc.scalar.activation(xt[:, xs], pf[:, :], AF.Relu)
                        else:
                            nc.vector.tensor_scalar_max(xt[:, xs], pf[:, :],
                                                        0.0)
                        if b == 0:
                            nc.scalar.activation(fcutT[0:42, cs], pf[0:42, :],
                                                 AF.Identity, scale=SF)
                        else:
                            nc.scalar.activation(fcutT[64:106, cs],
                                                 pf[64:106, :],
                                                 AF.Identity, scale=SF)
        if li > 0:
            for q in range(4 * (NB - 1), NQ):
                emit_tp(q, mm)
        # phase C: left^T = fcst.T @ adjT in fp8 DoubleRow, jt pairs of q.
        # li=0 uses pairs so compute can start before the whole adjacency
        # has landed in SBUF; li=3 uses pairs so the max-reduce drains
        # overlap the matmul stream instead of serializing at the tail.
        if li == 0 or last:
            groups = [(0, 2), (2, 4), (4, 6), (6, 8)]
        else:
            groups = [(0, 4), (4, 8)]
        for g0, g1 in groups:
            pls = {}
            for ib in range(g0, g1):
                pls[ib] = ps_left.tile([mm, 512], F32, tag="left",
                                       name=f"pl{li}_{ib}")
            for jt in range(NQ // 2):
                lhsT = fcst3[:, 2 * jt:2 * jt + 2, 0:mm]
                for ib in range(g0, g1):
                    rhs = adjs3[:, 2 * jt:2 * jt + 2,
                                ib * 512:(ib + 1) * 512]
                    nc.tensor.matmul(pls[ib][:], lhsT=lhsT, rhs=rhs,
                                     start=(jt == 0), stop=(jt == NQ // 2 - 1),
                                     perf_mode=DR)
            for ib in range(g0, g1):
                pl = pls[ib]
                if not last:
                    # xt keeps the (SA*SF)-scaled cut values; the next-layer
                    # gw cut rows are pre-divided on the host, and the gb cut
                    # biases are pre-multiplied (cols 8-10).
                    nc.vector.tensor_scalar(
                        xt[0:42, ib * 512:(ib + 1) * 512],
                        pl[0:42, :], bcol(8 + li, 0, 42), 0.0,
                        mybir.AluOpType.add, mybir.AluOpType.max)
                    nc.vector.tensor_scalar(
                        xt[64:106, N + ib * 512:N + (ib + 1) * 512],
                        pl[64:106, :], bcol(8 + li, 64, 106), 0.0,
                        mybir.AluOpType.add, mybir.AluOpType.max)
                else:
                    nc.vector.tensor_reduce(
                        mx[0:50, ib:ib + 1], pl[0:50, :],
                        mybir.AxisListType.X, mybir.AluOpType.max)
                    nc.vector.tensor_reduce(
                        mx[64:114, ib:ib + 1], pl[64:114, :],
                        mybir.AxisListType.X, mybir.AluOpType.max)

    # ---------------- final max + bias + output ----------------
    mxr = smallp.tile([128, 1], F32, tag="mxr")
    nc.vector.tensor_reduce(mxr[:], mx[:], mybir.AxisListType.X,
                            mybir.AluOpType.max)
    nc.scalar.activation(outsb[:], mxr[:], AF.Identity, bias=bcol(11, 0, 128),
                         scale=INV)
    nc.sync.dma_start(out=outd[:], in_=outsb[:])
    ctx.close()


# ---------------------------------------------------------------------------
# host side
# ---------------------------------------------------------------------------

# batch-1 feature permutation: rows 0:64 <- features 42:106,
# rows 64:106 <- features 0:42 (the cut), rows 106:128 <- features 106:128
PI = np.concatenate([np.arange(42, 106), np.arange(0, 42),
                     np.arange(106, 128)]).astype(np.int64)


def _prep_shared(inp):
    """Host preprocessing shared across cores (weights + adj)."""
    f32 = np.float32
    adjq = np.ascontiguousarray(
        inp["adj"].astype(f32).T * f32(SA)).astype(E4)

    gw0 = inp["gw0"].astype(f32)
    w3fold = (inp["pw3"].astype(f32) @ gw0[100:200])
    t4 = (inp["emb"].astype(f32) @ gw0[200:300])
    pb3f = (inp["pb3"].astype(f32) @ gw0[100:200]).astype(f32)
    gw0L = np.ascontiguousarray(gw0[:100])

    # pe_in row permutation: ours = [sin(f,c) x30 | cos(f,c) x30 | mesh x3]
    pw1f = inp["pw1"].astype(f32)
    pw1p_ = np.zeros((67, 25), f32)
    for k in range(30):
        f, c = divmod(k, 3)
        pw1p_[k] = pw1f[f * 6 + c]          # sin rows
        pw1p_[32 + k] = pw1f[f * 6 + 3 + c]  # cos rows
    pw1p_[64:67] = pw1f[60:63]
    pw1p = pw1p_.astype(BF)

    freqs = np.asarray([np.pi] + [2.0 * np.pi * i for i in range(1, 10)], f32)
    freq2 = np.repeat(freqs, 3) / (2.0 * np.pi)   # [30]
    self6 = np.zeros((6, 62), f32)
    for b in range(2):
        for k in range(30):
            self6[3 * b + k % 3, 32 * b + k] = freq2[k]

    # xt carries the adjacency-matmul output still scaled by SA*SF; undo the
    # scale by pre-dividing the gw rows that consume cut features (b0 rows
    # 0:42, permuted-b1 rows 64:106) and pre-multiplying the gb cut biases.
    gw1 = inp["gw1"].astype(f32)
    gw2 = inp["gw2"].astype(f32)
    gw3 = inp["gw3"].astype(f32)
    gw1p = gw1[np.ix_(PI, PI)].copy()
    gw2p = gw2[np.ix_(PI, PI)].copy()
    gw3p = gw3[PI, :].copy()
    for g in (gw1, gw2, gw3):
        g[0:42] *= f32(INV)
    for g in (gw1p, gw2p, gw3p):
        g[64:106] *= f32(INV)

    biasd = np.zeros((128, 12), f32)
    biasd[0:128, 0] = inp["ab1"][:128]
    biasd[0:72, 1] = inp["ab1"][128:200]
    biasd[0:100, 2] = inp["ab2"]
    biasd[0:100, 3] = inp["ab3"]
    biasd[0:128, 4] = pb3f
    biasd[0:128, 5] = pb3f[PI]
    biasd[0:25, 6] = inp["pb1"].astype(f32)
    biasd[0:50, 7] = inp["pb2"].astype(f32)
    for li in range(3):
        gb = inp[f"gb{li}"].astype(f32) * f32(SA * SF)
        biasd[0:42, 8 + li] = gb[:42]
        biasd[64:106, 8 + li] = gb[:42]
    gb3 = inp["gb3"].astype(f32)
    biasd[0:50, 11] = gb3
    biasd[64:114, 11] = gb3

    return {
        "adjq": adjq,
        "pw1p": pw1p,
        "pw2d": inp["pw2"].astype(BF),
        "w3fold": w3fold.astype(BF),
        "w3foldp": np.ascontiguousarray(w3fold[:, PI]).astype(BF),
        "t4d": t4.astype(BF),
        "t4pd": np.ascontiguousarray(t4[:, PI]).astype(BF),
        "gw1d": gw1.astype(BF),
        "gw1pd": np.ascontiguousarray(gw1p).astype(BF),
        "gw2d": gw2.astype(BF),
        "gw2pd": np.ascontiguousarray(gw2p).astype(BF),
        "gw3d": gw3.astype(BF),
        "gw3pd": np.ascontiguousarray(gw3p).astype(BF),
        "aw1ad": np.ascontiguousarray(inp["aw1"].astype(f32)[:, :128]),
        "aw1bd": np.ascontiguousarray(inp["aw1"].astype(f32)[:, 128:200]),
        "aw2ad": np.ascontiguousarray(inp["aw2"].astype(f32)[:128]),
        "aw2bd": np.ascontiguousarray(inp["aw2"].astype(f32)[128:200]),
        "aw3d": inp["aw3"].astype(f32),
        "gw0Ld": gw0L,
        "gw0Lpd": np.ascontiguousarray(gw0L[:, PI]),
        "selfd": self6,
        "biasd": biasd,
    }


def _prep_core(inp, shared, core):
    bs = slice(core * BC, (core + 1) * BC)
    f32 = np.float32
    mesh = inp["mesh"].astype(f32)[bs]                       # [2, N, 3]
    meshT = np.ascontiguousarray(mesh.transpose(0, 2, 1))    # [2, 3, N]
    mi = inp["mask_idx"][bs]                                 # [2, N] int32
    onehot = (mi[:, None, :] == np.arange(4, dtype=mi.dtype)[None, :, None])
    onehot = np.ascontiguousarray(
        onehot.transpose(1, 0, 2).reshape(4, BC * N)).astype(BF)
    maskT = np.ascontiguousarray(inp["mask"].astype(f32)[bs].T)  # [50, 2]
    m = dict(shared)
    m["meshTd"] = meshT
    m["meshTb"] = meshT.astype(BF)
    m["onehotd"] = onehot
    m["maskTd"] = maskT
    return m


_CACHED = {}


def kernel(**inputs) -> np.ndarray:
    if "nc" not in _CACHED:
        _CACHED["nc"] = build_bass()
    nc = _CACHED["nc"]
    shared = _prep_shared(inputs)
    in_maps = [_prep_core(inputs, shared, c) for c in range(NCORES)]
    res = run_bass_kernel_spmd(nc, in_maps, list(range(NCORES)), **run_kwargs)
    out = np.empty((B, 50), np.float32)
    for c in range(NCORES):
        o = res.results[c]["outd"][:, 0]
        out[2 * c] = o[0:50]
        out[2 * c + 1] = o[64:114]
    _CACHED["last_results"] = res
    return out
